# revision 9
# baseline (speedup 1.0000x reference)
"""Trainium2 Bass kernel for nn_KernelMachine (random-feature kernel machine).

Computes out = (sqrt(2/N) * cos(x @ Wf^T + bf)) @ Wp on 8 NeuronCores,
data-parallel over the batch dimension (1024 rows/core), no collectives.

Per-core device pipeline, per 128-wide tile of the N=4096 feature dim:
  1. TensorE (float32r): f' = x @ (Wf/2pi)^T          -> PSUM [128, 1024]
  2. VectorE custom op:  r = t - round(t), t = f'+bf'  (range reduce, turns)
  3. ScalarE:            g = Sin(2pi * r)              -> SBUF
  4. TensorE (float32r): out^T += Wp_tile^T @ g        (PSUM accumulate)

Host-side prep: transpose/scale/pack the small weight tensors; the cos->sin
shift (pi/2), the positivity offset (+16 turns), and the sqrt(2/N) scale are
folded into bf' and Wp.
"""
import sys

if "/opt/trn_rl_repo" not in sys.path:
    sys.path.insert(0, "/opt/trn_rl_repo")

import numpy as np

import concourse.bacc as bacc
import concourse.mybir as mybir
import concourse.tile as tile
from concourse import bass_utils
from concourse import dve_ops
from concourse.dve_spec import C0, C1, Spec, Src0, lower
from concourse.dve_uop import DveOpSpec

# Problem shape (hardcoded per contest contract).
B = 8192
D = 64
N = 4096
M = 8
NCORES = 8
BS = B // NCORES  # 1024 batch rows per core
P = 128
NT = N // P  # 32 feature tiles
FREE = 512  # matmul moving free dim

f32 = mybir.dt.float32
f32r = mybir.dt.float32r

MAGIC = float(2**23)  # fp32 round-to-nearest-int magic constant
TWO_PI = float(2 * np.pi)


def _make_frac_op():
    """Custom DVE op: out = t - rne_round(t) with t = in0 + s0; s1 must be 2^23.

    Registered into concourse.dve_ops at import time (idempotent)."""
    name = "FRAC_WRAP_ANT"
    for o in dve_ops.OPS:
        if o.name == name:
            return o
    t = Src0 + C0
    body = t - ((t + C1) - C1)

    def ref(in0, in1, s0, s1, imm2):
        tt = (in0.astype(np.float32) + np.float32(s0)).astype(np.float32)
        u = ((tt + np.float32(s1)).astype(np.float32) - np.float32(s1)).astype(
            np.float32
        )
        return (tt - u).astype(np.float32)

    spec = Spec(body=body, reference=ref)
    opcode = dve_ops._CUSTOM_DVE_ROW_BASE + len(dve_ops.OPS)
    dve_ops._SUB_OPCODE_FOR_NAME[name] = opcode
    shas = {}
    for ver in ("v3", "v4"):
        tmp = DveOpSpec(
            name=name, opcode=opcode, uops=lower(spec, ver=ver), rd1_en=False
        )
        shas[ver] = tmp.sha(ver)
    op = dve_ops.DveOp(name, spec, subdim=False, uops_sha=shas)
    dve_ops.OPS.append(op)
    dve_ops.CUSTOM_DVE_SPECS[name] = spec
    return op


def build(fpsum_bufs=3, gpool_bufs=3, pipeline_mm2=False, split_dve=1,
          skip_dve=False, skip_act=False, skip_mm2=False):
    frac_op = _make_frac_op()
    nc = bacc.Bacc("TRN2", target_bir_lowering=False, debug=False, num_devices=NCORES)

    xt_d = nc.dram_tensor("xt", [D, BS], f32, kind="ExternalInput").ap()
    wft_d = nc.dram_tensor("wft", [D, N], f32, kind="ExternalInput").ap()
    bfp_d = nc.dram_tensor("bfp", [P, NT], f32, kind="ExternalInput").ap()
    wps_d = nc.dram_tensor("wps", [P, NT, M], f32, kind="ExternalInput").ap()
    out_d = nc.dram_tensor("out", [M, BS], f32, kind="ExternalOutput").ap()

    with tile.TileContext(nc) as tc:
        with (
            tc.tile_pool(name="singles", bufs=1) as singles,
            tc.tile_pool(name="gpool", bufs=gpool_bufs) as gpool,
            tc.tile_pool(name="fpsum", bufs=fpsum_bufs, space="PSUM") as fpsum,
            tc.tile_pool(name="opsum", bufs=1, space="PSUM") as opsum_pool,
        ):
            # Preload everything (1.45 MB total); chunked so DMA queues parallelize
            # and the first matmuls can start before all chunks land.
            wft_tiles = []
            for c in range(8):
                t_ = singles.tile([D, FREE], f32r, tag=f"wft{c}")
                nc.sync.dma_start(t_, wft_d[:, c * FREE:(c + 1) * FREE].bitcast(f32r))
                wft_tiles.append(t_)
            xt_tiles = []
            for j in range(2):
                t_ = singles.tile([D, FREE], f32r, tag=f"xt{j}")
                nc.sync.dma_start(t_, xt_d[:, j * FREE:(j + 1) * FREE].bitcast(f32r))
                xt_tiles.append(t_)
            bfp_sb = singles.tile([P, NT], f32, tag="bfp")
            nc.sync.dma_start(bfp_sb, bfp_d)
            wps_sb = singles.tile([P, NT, M], f32r, tag="wps")
            nc.sync.dma_start(wps_sb, wps_d.bitcast(f32r))
            out_sb = singles.tile([M, BS], f32, tag="outsb")

            opsum = None if skip_mm2 else opsum_pool.tile([M, BS], f32)

            def emit_mm2(t, gt):
                for j in range(2):
                    nc.tensor.matmul(
                        opsum[:, j * FREE:(j + 1) * FREE],
                        lhsT=wps_sb[:, t],
                        rhs=gt[:, j * FREE:(j + 1) * FREE],
                        start=(t == 0),
                        stop=(t == NT - 1),
                    )

            pending = None  # (t, gt) waiting for its mm2 (software-pipelined)
            for t in range(NT):
                fps = fpsum.tile([P, BS], f32)
                lhsT = wft_tiles[t // 4][:, (t % 4) * P:(t % 4 + 1) * P]
                for j in range(2):
                    nc.tensor.matmul(
                        fps[:, j * FREE:(j + 1) * FREE],
                        lhsT=lhsT,
                        rhs=xt_tiles[j][:],
                        start=True,
                        stop=True,
                    )
                if pipeline_mm2 and pending is not None:
                    emit_mm2(*pending)
                    pending = None
                if not skip_dve:
                    for h in range(split_dve):
                        W = BS // split_dve
                        nc.vector._custom_dve(
                            frac_op,
                            out=fps[:, h * W:(h + 1) * W],
                            in0=fps[:, h * W:(h + 1) * W],
                            s0=bfp_sb[:, t:t + 1],
                            s1=MAGIC,
                        )
                last_fps = fps
                if not (skip_act and skip_mm2):
                    gt = gpool.tile([P, BS], f32r)
                    if not skip_act:
                        nc.scalar.activation(
                            gt[:], fps[:], mybir.ActivationFunctionType.Sin,
                            scale=TWO_PI,
                        )
                    else:
                        nc.vector.tensor_copy(out=gt[:], in_=fps[:])
                    if skip_mm2:
                        pass
                    elif pipeline_mm2:
                        pending = (t, gt)
                    else:
                        emit_mm2(t, gt)
            if pending is not None:
                emit_mm2(*pending)
            if skip_mm2:
                nc.any.tensor_copy(out=out_sb[:], in_=last_fps[:M, :])
            else:
                nc.any.tensor_copy(out=out_sb[:], in_=opsum[:])
            nc.sync.dma_start(out_d, out_sb[:])
    nc.compile()
    return nc


_NC = None


def _get_nc():
    global _NC
    if _NC is None:
        _NC = build()
    return _NC


def _prep_in_maps(x, Wf, bf, Wp):
    scale = np.sqrt(np.float32(2.0) / np.float32(N))
    inv2pi = np.float64(1.0) / (2.0 * np.pi)
    wft = np.ascontiguousarray(
        (Wf.astype(np.float64) * inv2pi).astype(np.float32).T
    )  # [64, 4096]
    # bf' in turns: cos(u) = sin(u + pi/2); +16 keeps t positive (|f'| < 8 turns)
    bfp = np.ascontiguousarray(
        ((bf.astype(np.float64) + np.pi / 2) * inv2pi + 16.0)
        .astype(np.float32)
        .reshape(NT, P)
        .T
    )  # [128, 32]
    wps = np.ascontiguousarray(
        (scale * Wp.astype(np.float32)).reshape(NT, P, M).transpose(1, 0, 2)
    )  # [128, 32, 8]
    in_maps = []
    for c in range(NCORES):
        xt = np.ascontiguousarray(x[c * BS:(c + 1) * BS].astype(np.float32).T)
        in_maps.append({"xt": xt, "wft": wft, "bfp": bfp, "wps": wps})
    return in_maps


def run(x, Wf, bf, Wp, trace=False):
    nc = _get_nc()
    in_maps = _prep_in_maps(x, Wf, bf, Wp)
    res = bass_utils.run_bass_kernel_spmd(
        nc, in_maps, core_ids=list(range(NCORES)), trace=trace
    )
    out = np.empty((B, M), dtype=np.float32)
    for c in range(NCORES):
        out[c * BS:(c + 1) * BS, :] = res.results[c]["out"].T
    return out, res


def kernel(x, Wf, bf, Wp):
    x = np.asarray(x)
    Wf = np.asarray(Wf)
    bf = np.asarray(bf)
    Wp = np.asarray(Wp)
    out, _ = run(x, Wf, bf, Wp, trace=False)
    return out
